# revision 1
# baseline (speedup 1.0000x reference)
"""Categorical cross-entropy loss kernel for Trainium2 (8 NeuronCores).

Computes: out = [-sum(input * log(target + 1e-8)) / B] for input/target of
shape [B=262144, C=128] float32.

Strategy (data-parallel, memory-bound streaming reduction):
  - Shard both tensors along batch across 8 cores (32768 rows each).
  - Each core views its [32768, 128] shard as [128 partitions, 32768 free]
    (partition p owns 256 contiguous rows -> contiguous 128 KiB per
    partition), streams it in 8 chunks of [128, 4096] (2 MiB DMAs).
  - Per chunk: ACT computes log(target + eps) in place, then one fused DVE
    TensorTensorReduce computes input * log_t and its per-partition sum.
  - Per-core output: [128, 8] partial sums; host sums in float64, scales
    by -1/B.
"""

import numpy as np

import concourse.bass as bass
import concourse.tile as tile
from concourse import bacc, mybir
from concourse.bass_utils import run_bass_kernel_spmd

B, C = 262144, 128
NCORES = 8
ROWS = B // NCORES          # 32768 rows per core
P = 128                     # SBUF partitions
FREE = ROWS * C // P        # 32768 f32 per partition
EPS = 1e-8

_NC_CACHE = None


# body chunks stream at full DMA width; the tapered tail shrinks the
# serial ACT->DVE chain after the last byte lands (geometric taper to a
# 128-elem final chunk = 512B/partition, the DMA line-rate threshold)
CH_SCHEDULE = [4096] * 6 + [2048] * 3 + [1024, 512, 256, 128, 128]
assert sum(CH_SCHEDULE) == FREE


def build_nc(repeat: int = 1, ch_schedule=None, io_bufs: int = 3,
             scratch_bufs: int = 3, inplace_mult: bool = False,
             alt_dma: bool = False, split_rings: bool = False,
             compute: str = "full", act_oop: bool = True,
             lean_preamble: bool = True, warmup_dma: bool = False) -> bass.Bass:
    if ch_schedule is None:
        ch_schedule = CH_SCHEDULE
    assert sum(ch_schedule) == FREE
    nch = len(ch_schedule)
    offs = [0]
    for c in ch_schedule:
        offs.append(offs[-1] + c)
    max_ch = max(ch_schedule)
    nc = bacc.Bacc("TRN2", target_bir_lowering=False, debug=False,
                   num_devices=NCORES)
    if lean_preamble:
        # Bass.__init__ memsets 4 const APs (0.0/1.0 f32, 1.0 bf16, 127 u8)
        # on gpsimd before the init barrier; nothing in this kernel reads
        # them (the eps bias is our own tile), so drop the serial memsets.
        # The barrier instructions stay -- removal only unwrites tensors
        # that have no readers, so it cannot introduce a race.
        bb = nc.cur_bb.bb
        bb.instructions = [
            i for i in bb.instructions
            if not (isinstance(i, mybir.InstMemset)
                    and i.outs and "const-" in str(i.outs[0]))
        ]
    inp = nc.dram_tensor("input", [ROWS, C], mybir.dt.float32,
                         kind="ExternalInput").ap()
    tgt = nc.dram_tensor("target", [ROWS, C], mybir.dt.float32,
                         kind="ExternalInput").ap()
    out = nc.dram_tensor("out", [P, nch], mybir.dt.float32,
                         kind="ExternalOutput").ap()

    inp_v = inp.rearrange("(p n) c -> p (n c)", p=P)
    tgt_v = tgt.rearrange("(p n) c -> p (n c)", p=P)

    with tile.TileContext(nc) as tc:
        with (
            tc.tile_pool(name="eps", bufs=1) as eps_pool,
            tc.tile_pool(name="io", bufs=io_bufs) as io_pool,
            tc.tile_pool(name="scratch", bufs=scratch_bufs) as scratch_pool,
            tc.tile_pool(name="acc", bufs=1) as acc_pool,
        ):
            # EPS bias for the ACT Ln; Tile tracks the memset->ACT dep so
            # it overlaps the first DMAs (no extra all-engine barrier)
            if compute != "none":
                eps_t = eps_pool.tile([P, 1], mybir.dt.float32)
                nc.gpsimd.memset(eps_t[:], EPS)
            if warmup_dma:
                wt = eps_pool.tile([P, 1], mybir.dt.float32, tag="warm")
                nc.sync.dma_start(wt[:], inp_v[:, 0:1])
                nc.vector.tensor_copy(wt[:], wt[:])  # keep a reader

            acc = None
            if compute == "full":
                acc = acc_pool.tile([P, nch], mybir.dt.float32)
            last_tt = None
            for it in range(nch * repeat):
                j = it % nch
                ch = ch_schedule[j]
                dma = nc.scalar if (alt_dma and it % 2) else nc.sync
                if split_rings == "gpsimd":
                    dma_inp = nc.gpsimd
                elif split_rings:
                    dma_inp = nc.scalar
                else:
                    dma_inp = dma
                # target first: ACT only needs tgt, so it can start while
                # input is still in flight
                tt = io_pool.tile([P, max_ch], mybir.dt.float32, tag="tgt")
                dma.dma_start(tt[:, :ch], tgt_v[:, offs[j]:offs[j] + ch])
                ti = io_pool.tile([P, max_ch], mybir.dt.float32, tag="inp")
                dma_inp.dma_start(ti[:, :ch], inp_v[:, offs[j]:offs[j] + ch])
                last_tt = tt
                if compute == "none":
                    continue
                if act_oop:
                    # log into scratch: tt's buffer frees right after ACT
                    # reads it, giving tgt DMAs one more stage of lead time
                    prod = scratch_pool.tile([P, max_ch], mybir.dt.float32)
                    nc.scalar.activation(prod[:, :ch], tt[:, :ch],
                                         mybir.ActivationFunctionType.Ln,
                                         bias=eps_t[:])
                    if compute == "act":
                        continue
                    nc.vector.tensor_tensor(prod[:, :ch], ti[:, :ch],
                                            prod[:, :ch],
                                            mybir.AluOpType.mult)
                    nc.vector.tensor_reduce(acc[:, j:j + 1], prod[:, :ch],
                                            mybir.AxisListType.X,
                                            mybir.AluOpType.add)
                    continue
                # tt = log(tt + EPS), in place on the ACT engine
                nc.scalar.activation(tt[:, :ch], tt[:, :ch],
                                     mybir.ActivationFunctionType.Ln,
                                     bias=eps_t[:])
                if compute == "act":
                    continue
                # acc[:, j] = sum_free(ti * tt)
                # (TensorTensorReduce would fuse these, but it crashes the
                # device on this runtime build -- use 2 DVE ops instead)
                if inplace_mult:
                    prod = ti
                else:
                    prod = scratch_pool.tile([P, max_ch], mybir.dt.float32)
                nc.vector.tensor_tensor(prod[:, :ch], ti[:, :ch], tt[:, :ch],
                                        mybir.AluOpType.mult)
                nc.vector.tensor_reduce(acc[:, j:j + 1], prod[:, :ch],
                                        mybir.AxisListType.X,
                                        mybir.AluOpType.add)
            if compute == "full":
                nc.sync.dma_start(out[:], acc[:])
            else:  # timing probes: output is garbage, deps only on last tile
                nc.sync.dma_start(out[:], last_tt[:, :nch])
    nc.compile()
    return nc


def shard_inputs(inp: np.ndarray, tgt: np.ndarray) -> list[dict]:
    return [
        {
            "input": np.ascontiguousarray(inp[i * ROWS:(i + 1) * ROWS]),
            "target": np.ascontiguousarray(tgt[i * ROWS:(i + 1) * ROWS]),
        }
        for i in range(NCORES)
    ]


def combine(results: list[dict]) -> np.ndarray:
    total = 0.0
    for r in results:
        total += float(np.sum(np.asarray(r["out"], dtype=np.float64)))
    return np.array([-total / B], dtype=np.float32)


def kernel(**inputs: np.ndarray) -> np.ndarray:
    global _NC_CACHE
    inp = np.ascontiguousarray(np.asarray(inputs["input"], dtype=np.float32))
    tgt = np.ascontiguousarray(np.asarray(inputs["target"], dtype=np.float32))
    assert inp.shape == (B, C) and tgt.shape == (B, C)

    if _NC_CACHE is None:
        _NC_CACHE = build_nc()
    nc = _NC_CACHE

    res = run_bass_kernel_spmd(nc, shard_inputs(inp, tgt),
                               list(range(NCORES)))
    return combine(res.results)



# revision 8
# speedup vs baseline: 1.0826x; 1.0826x over previous
"""Categorical cross-entropy loss kernel for Trainium2 (8 NeuronCores).

Computes: out = [-sum(input * log(target + 1e-8)) / B] for input/target of
shape [B=262144, C=128] float32.

Strategy (data-parallel, memory-bound streaming reduction):
  - Shard both tensors along batch across 8 cores (32768 rows each).
  - Each core views its [32768, 128] shard as [128 partitions, 32768 free]
    (partition p owns 256 contiguous rows -> contiguous 128 KiB per
    partition), streams it in 8 uniform chunks of [128, 4096] (2 MiB DMAs,
    16 KiB contiguous per partition = SDMA line rate; measured ~408 GB/s
    per core, DMA-bound with compute fully hidden).
  - Per chunk: ACT computes log(target + eps) out of place, then DVE
    multiplies by input and reduces over the free axis into acc[:, j].
  - Per-core output: [128, 8] partial sums; host sums in float64, scales
    by -1/B.
"""

import numpy as np

import concourse.bass as bass
import concourse.tile as tile
from concourse import bacc, mybir
from concourse.bass_utils import run_bass_kernel_spmd

B, C = 262144, 128
NCORES = 8
ROWS = B // NCORES          # 32768 rows per core
P = 128                     # SBUF partitions
FREE = ROWS * C // P        # 32768 f32 per partition
EPS = 1e-8

_NC_CACHE = None


# Uniform full-width chunks: every DMA is [128, 4096] f32 = 2 MiB with
# 16 KiB contiguous per partition -- max SDMA line rate.  A tapered tail
# (4096..128) was measured 10 us/pass SLOWER in steady state: the small
# trailing chunks pay the sub-1MiB DMA efficiency cliff on every pass,
# which outweighs the shorter single-pass drain they were added for.
CH_SCHEDULE = [4096] * 8
assert sum(CH_SCHEDULE) == FREE


def build_nc(repeat: int = 1, ch_schedule=None, io_bufs: int = 3,
             scratch_bufs: int = 3, inplace_mult: bool = False,
             alt_dma: bool = False, split_rings: bool = False,
             compute: str = "full", act_oop: bool = True,
             lean_preamble: bool = True, warmup_dma: bool = False,
             layout: str = "strided") -> bass.Bass:
    if ch_schedule is None:
        ch_schedule = CH_SCHEDULE
    assert sum(ch_schedule) == FREE
    nch = len(ch_schedule)
    offs = [0]
    for c in ch_schedule:
        offs.append(offs[-1] + c)
    max_ch = max(ch_schedule)
    if layout == "seq":
        # chunk j = contiguous DRAM range [j*ch*P*4, ...): partition p owns
        # the p-th (ch*4)-byte span of it.  Requires uniform chunks whose
        # row count (=ch) splits into whole rows per partition (ch%P==0).
        assert len(set(ch_schedule)) == 1 and ch_schedule[0] % P == 0
    nc = bacc.Bacc("TRN2", target_bir_lowering=False, debug=False,
                   num_devices=NCORES)
    if lean_preamble:
        # Bass.__init__ memsets 4 const APs (0.0/1.0 f32, 1.0 bf16, 127 u8)
        # on gpsimd before the init barrier; nothing in this kernel reads
        # them (the eps bias is our own tile), so drop the serial memsets.
        # The barrier instructions stay -- removal only unwrites tensors
        # that have no readers, so it cannot introduce a race.
        bb = nc.cur_bb.bb
        bb.instructions = [
            i for i in bb.instructions
            if not (isinstance(i, mybir.InstMemset)
                    and i.outs and "const-" in str(i.outs[0]))
        ]
    inp = nc.dram_tensor("input", [ROWS, C], mybir.dt.float32,
                         kind="ExternalInput").ap()
    tgt = nc.dram_tensor("target", [ROWS, C], mybir.dt.float32,
                         kind="ExternalInput").ap()
    out = nc.dram_tensor("out", [P, nch], mybir.dt.float32,
                         kind="ExternalOutput").ap()

    if layout == "seq":
        inp_v3 = inp.rearrange("(j p n) c -> p j (n c)", p=P, j=nch)
        tgt_v3 = tgt.rearrange("(j p n) c -> p j (n c)", p=P, j=nch)
        inp_src = lambda j, ch: inp_v3[:, j]
        tgt_src = lambda j, ch: tgt_v3[:, j]
    else:
        inp_v = inp.rearrange("(p n) c -> p (n c)", p=P)
        tgt_v = tgt.rearrange("(p n) c -> p (n c)", p=P)
        inp_src = lambda j, ch: inp_v[:, offs[j]:offs[j] + ch]
        tgt_src = lambda j, ch: tgt_v[:, offs[j]:offs[j] + ch]

    with tile.TileContext(nc) as tc:
        with (
            tc.tile_pool(name="eps", bufs=1) as eps_pool,
            tc.tile_pool(name="io", bufs=io_bufs) as io_pool,
            tc.tile_pool(name="scratch", bufs=scratch_bufs) as scratch_pool,
            tc.tile_pool(name="acc", bufs=1) as acc_pool,
        ):
            # EPS bias for the ACT Ln; Tile tracks the memset->ACT dep so
            # it overlaps the first DMAs (no extra all-engine barrier)
            if compute != "none":
                eps_t = eps_pool.tile([P, 1], mybir.dt.float32)
                nc.gpsimd.memset(eps_t[:], EPS)
            if warmup_dma:
                wt = eps_pool.tile([P, 1], mybir.dt.float32, tag="warm")
                nc.sync.dma_start(wt[:], inp_src(0, max_ch)[:, 0:1])
                nc.vector.tensor_copy(wt[:], wt[:])  # keep a reader

            acc = None
            if compute == "full":
                acc = acc_pool.tile([P, nch], mybir.dt.float32)
            last_tt = None
            for it in range(nch * repeat):
                j = it % nch
                ch = ch_schedule[j]
                dma = nc.scalar if (alt_dma and it % 2) else nc.sync
                if split_rings == "gpsimd":
                    dma_inp = nc.gpsimd
                elif split_rings:
                    dma_inp = nc.scalar
                else:
                    dma_inp = dma
                # target first: ACT only needs tgt, so it can start while
                # input is still in flight
                tt = io_pool.tile([P, max_ch], mybir.dt.float32, tag="tgt")
                dma.dma_start(tt[:, :ch], tgt_src(j, ch))
                ti = io_pool.tile([P, max_ch], mybir.dt.float32, tag="inp")
                dma_inp.dma_start(ti[:, :ch], inp_src(j, ch))
                last_tt = tt
                if compute == "none":
                    continue
                if act_oop:
                    # log into scratch: tt's buffer frees right after ACT
                    # reads it, giving tgt DMAs one more stage of lead time
                    prod = scratch_pool.tile([P, max_ch], mybir.dt.float32)
                    nc.scalar.activation(prod[:, :ch], tt[:, :ch],
                                         mybir.ActivationFunctionType.Ln,
                                         bias=eps_t[:])
                    if compute == "act":
                        continue
                    nc.vector.tensor_tensor(prod[:, :ch], ti[:, :ch],
                                            prod[:, :ch],
                                            mybir.AluOpType.mult)
                    nc.vector.tensor_reduce(acc[:, j:j + 1], prod[:, :ch],
                                            mybir.AxisListType.X,
                                            mybir.AluOpType.add)
                    continue
                # tt = log(tt + EPS), in place on the ACT engine
                nc.scalar.activation(tt[:, :ch], tt[:, :ch],
                                     mybir.ActivationFunctionType.Ln,
                                     bias=eps_t[:])
                if compute == "act":
                    continue
                # acc[:, j] = sum_free(ti * tt)
                # (TensorTensorReduce would fuse these, but it crashes the
                # device on this runtime build -- use 2 DVE ops instead)
                if inplace_mult:
                    prod = ti
                else:
                    prod = scratch_pool.tile([P, max_ch], mybir.dt.float32)
                nc.vector.tensor_tensor(prod[:, :ch], ti[:, :ch], tt[:, :ch],
                                        mybir.AluOpType.mult)
                nc.vector.tensor_reduce(acc[:, j:j + 1], prod[:, :ch],
                                        mybir.AxisListType.X,
                                        mybir.AluOpType.add)
            if compute == "full":
                nc.sync.dma_start(out[:], acc[:])
            else:  # timing probes: output is garbage, deps only on last tile
                nc.sync.dma_start(out[:], last_tt[:, :nch])
    nc.compile()
    return nc


def shard_inputs(inp: np.ndarray, tgt: np.ndarray) -> list[dict]:
    return [
        {
            "input": np.ascontiguousarray(inp[i * ROWS:(i + 1) * ROWS]),
            "target": np.ascontiguousarray(tgt[i * ROWS:(i + 1) * ROWS]),
        }
        for i in range(NCORES)
    ]


def combine(results: list[dict]) -> np.ndarray:
    total = 0.0
    for r in results:
        total += float(np.sum(np.asarray(r["out"], dtype=np.float64)))
    return np.array([-total / B], dtype=np.float32)


def kernel(**inputs: np.ndarray) -> np.ndarray:
    global _NC_CACHE
    inp = np.ascontiguousarray(np.asarray(inputs["input"], dtype=np.float32))
    tgt = np.ascontiguousarray(np.asarray(inputs["target"], dtype=np.float32))
    assert inp.shape == (B, C) and tgt.shape == (B, C)

    if _NC_CACHE is None:
        _NC_CACHE = build_nc()
    nc = _NC_CACHE

    res = run_bass_kernel_spmd(nc, shard_inputs(inp, tgt),
                               list(range(NCORES)))
    return combine(res.results)



# revision 10
# speedup vs baseline: 1.1509x; 1.0632x over previous
"""Categorical cross-entropy loss kernel for Trainium2 (8 NeuronCores).

Computes: out = [-sum(input * log(target + 1e-8)) / B] for input/target of
shape [B=262144, C=128] float32.

Strategy (data-parallel, memory-bound streaming reduction):
  - Shard both tensors along batch across 8 cores (32768 rows each).
  - Each core views its [32768, 128] shard as [128 partitions, 32768 free]
    (partition p owns 256 contiguous rows -> contiguous 128 KiB per
    partition), streams it in 8 uniform chunks of [128, 4096] (2 MiB DMAs,
    16 KiB contiguous per partition = SDMA line rate; measured ~408 GB/s
    per core, DMA-bound with compute fully hidden).
  - Per chunk: ACT computes log(target + eps) out of place, then DVE
    multiplies by input and reduces over the free axis into acc[:, j].
  - Per-core output: [128, 8] partial sums; host sums in float64, scales
    by -1/B.
"""

import numpy as np

import concourse.bass as bass
import concourse.tile as tile
from concourse import bacc, mybir
from concourse.bass_utils import run_bass_kernel_spmd

B, C = 262144, 128
NCORES = 8
ROWS = B // NCORES          # 32768 rows per core
P = 128                     # SBUF partitions
FREE = ROWS * C // P        # 32768 f32 per partition
EPS = 1e-8

_NC_CACHE = None


# Uniform full-width chunks: every DMA is [128, 4096] f32 = 2 MiB with
# 16 KiB contiguous per partition -- max SDMA line rate.  A tapered tail
# (4096..128) was measured 10 us/pass SLOWER in steady state: the small
# trailing chunks pay the sub-1MiB DMA efficiency cliff on every pass,
# which outweighs the shorter single-pass drain they were added for.
CH_SCHEDULE = [4096] * 8
assert sum(CH_SCHEDULE) == FREE


def build_nc(repeat: int = 1, ch_schedule=None, io_bufs: int = 3,
             scratch_bufs: int = 3, inplace_mult: bool = False,
             alt_dma: bool = False, split_rings: bool = False,
             compute: str = "full", act_oop: bool = True,
             lean_preamble: bool = True, warmup_dma: bool = False,
             layout: str = "strided") -> bass.Bass:
    if ch_schedule is None:
        ch_schedule = CH_SCHEDULE
    assert sum(ch_schedule) == FREE
    nch = len(ch_schedule)
    offs = [0]
    for c in ch_schedule:
        offs.append(offs[-1] + c)
    max_ch = max(ch_schedule)
    if layout == "seq":
        # chunk j = contiguous DRAM range [j*ch*P*4, ...): partition p owns
        # the p-th (ch*4)-byte span of it.  Requires uniform chunks whose
        # row count (=ch) splits into whole rows per partition (ch%P==0).
        assert len(set(ch_schedule)) == 1 and ch_schedule[0] % P == 0
    nc = bacc.Bacc("TRN2", target_bir_lowering=False, debug=False,
                   num_devices=NCORES)
    if lean_preamble:
        # Bass.__init__ memsets 4 const APs (0.0/1.0 f32, 1.0 bf16, 127 u8)
        # on gpsimd before the init barrier; nothing in this kernel reads
        # them (the eps bias is our own tile), so drop the serial memsets.
        # The barrier instructions stay -- removal only unwrites tensors
        # that have no readers, so it cannot introduce a race.
        bb = nc.cur_bb.bb
        bb.instructions = [
            i for i in bb.instructions
            if not (isinstance(i, mybir.InstMemset)
                    and i.outs and "const-" in str(i.outs[0]))
        ]
    inp = nc.dram_tensor("input", [ROWS, C], mybir.dt.float32,
                         kind="ExternalInput").ap()
    tgt = nc.dram_tensor("target", [ROWS, C], mybir.dt.float32,
                         kind="ExternalInput").ap()
    out = nc.dram_tensor("out", [P, nch], mybir.dt.float32,
                         kind="ExternalOutput").ap()

    if layout == "seq":
        inp_v3 = inp.rearrange("(j p n) c -> p j (n c)", p=P, j=nch)
        tgt_v3 = tgt.rearrange("(j p n) c -> p j (n c)", p=P, j=nch)
        inp_src = lambda j, ch: inp_v3[:, j]
        tgt_src = lambda j, ch: tgt_v3[:, j]
    else:
        inp_v = inp.rearrange("(p n) c -> p (n c)", p=P)
        tgt_v = tgt.rearrange("(p n) c -> p (n c)", p=P)
        inp_src = lambda j, ch: inp_v[:, offs[j]:offs[j] + ch]
        tgt_src = lambda j, ch: tgt_v[:, offs[j]:offs[j] + ch]

    with tile.TileContext(nc) as tc:
        with (
            tc.tile_pool(name="eps", bufs=1) as eps_pool,
            tc.tile_pool(name="io", bufs=io_bufs) as io_pool,
            tc.tile_pool(name="scratch", bufs=scratch_bufs) as scratch_pool,
            tc.tile_pool(name="acc", bufs=1) as acc_pool,
        ):
            # EPS bias for the ACT Ln; Tile tracks the memset->ACT dep so
            # it overlaps the first DMAs (no extra all-engine barrier)
            if compute != "none":
                eps_t = eps_pool.tile([P, 1], mybir.dt.float32)
                nc.gpsimd.memset(eps_t[:], EPS)
            if warmup_dma:
                wt = eps_pool.tile([P, 1], mybir.dt.float32, tag="warm")
                nc.sync.dma_start(wt[:], inp_src(0, max_ch)[:, 0:1])
                nc.vector.tensor_copy(wt[:], wt[:])  # keep a reader

            acc = None
            if compute == "full":
                acc = acc_pool.tile([P, nch], mybir.dt.float32)
            last_tt = None
            for it in range(nch * repeat):
                j = it % nch
                ch = ch_schedule[j]
                if alt_dma == "cross":
                    # both HWDGE rings busy every chunk, each carrying one
                    # tensor, swapping per chunk so neither ring owns a
                    # tensor's full stream
                    dma = nc.sync if it % 2 == 0 else nc.scalar
                elif alt_dma == "crossg":
                    # like cross, but the second issue stream is gpsimd
                    # SWDGE (otherwise idle) instead of the ACT/scalar
                    # queue, which also carries the Ln activations
                    dma = nc.sync if it % 2 == 0 else nc.gpsimd
                else:
                    dma = nc.scalar if (alt_dma and it % 2) else nc.sync
                if alt_dma == "cross":
                    dma_inp = nc.scalar if it % 2 == 0 else nc.sync
                elif alt_dma == "crossg":
                    dma_inp = nc.gpsimd if it % 2 == 0 else nc.sync
                elif split_rings == "gpsimd":
                    dma_inp = nc.gpsimd
                elif split_rings:
                    dma_inp = nc.scalar
                else:
                    dma_inp = dma
                # target first: ACT only needs tgt, so it can start while
                # input is still in flight
                tt = io_pool.tile([P, max_ch], mybir.dt.float32, tag="tgt")
                dma.dma_start(tt[:, :ch], tgt_src(j, ch))
                ti = io_pool.tile([P, max_ch], mybir.dt.float32, tag="inp")
                dma_inp.dma_start(ti[:, :ch], inp_src(j, ch))
                last_tt = tt
                if compute == "none":
                    continue
                if act_oop:
                    # log into scratch: tt's buffer frees right after ACT
                    # reads it, giving tgt DMAs one more stage of lead time
                    prod = scratch_pool.tile([P, max_ch], mybir.dt.float32)
                    nc.scalar.activation(prod[:, :ch], tt[:, :ch],
                                         mybir.ActivationFunctionType.Ln,
                                         bias=eps_t[:])
                    if compute == "act":
                        continue
                    nc.vector.tensor_tensor(prod[:, :ch], ti[:, :ch],
                                            prod[:, :ch],
                                            mybir.AluOpType.mult)
                    nc.vector.tensor_reduce(acc[:, j:j + 1], prod[:, :ch],
                                            mybir.AxisListType.X,
                                            mybir.AluOpType.add)
                    continue
                # tt = log(tt + EPS), in place on the ACT engine
                nc.scalar.activation(tt[:, :ch], tt[:, :ch],
                                     mybir.ActivationFunctionType.Ln,
                                     bias=eps_t[:])
                if compute == "act":
                    continue
                # acc[:, j] = sum_free(ti * tt)
                # (TensorTensorReduce would fuse these, but it crashes the
                # device on this runtime build -- use 2 DVE ops instead)
                if inplace_mult:
                    prod = ti
                else:
                    prod = scratch_pool.tile([P, max_ch], mybir.dt.float32)
                nc.vector.tensor_tensor(prod[:, :ch], ti[:, :ch], tt[:, :ch],
                                        mybir.AluOpType.mult)
                nc.vector.tensor_reduce(acc[:, j:j + 1], prod[:, :ch],
                                        mybir.AxisListType.X,
                                        mybir.AluOpType.add)
            if compute == "full":
                nc.sync.dma_start(out[:], acc[:])
            else:  # timing probes: output is garbage, deps only on last tile
                nc.sync.dma_start(out[:], last_tt[:, :nch])
    nc.compile()
    return nc


def shard_inputs(inp: np.ndarray, tgt: np.ndarray) -> list[dict]:
    return [
        {
            "input": np.ascontiguousarray(inp[i * ROWS:(i + 1) * ROWS]),
            "target": np.ascontiguousarray(tgt[i * ROWS:(i + 1) * ROWS]),
        }
        for i in range(NCORES)
    ]


def combine(results: list[dict]) -> np.ndarray:
    total = 0.0
    for r in results:
        total += float(np.sum(np.asarray(r["out"], dtype=np.float64)))
    return np.array([-total / B], dtype=np.float32)


def kernel(**inputs: np.ndarray) -> np.ndarray:
    global _NC_CACHE
    inp = np.ascontiguousarray(np.asarray(inputs["input"], dtype=np.float32))
    tgt = np.ascontiguousarray(np.asarray(inputs["target"], dtype=np.float32))
    assert inp.shape == (B, C) and tgt.shape == (B, C)

    if _NC_CACHE is None:
        _NC_CACHE = build_nc()
    nc = _NC_CACHE

    res = run_bass_kernel_spmd(nc, shard_inputs(inp, tgt),
                               list(range(NCORES)))
    return combine(res.results)

